# revision 17
# baseline (speedup 1.0000x reference)
"""Causal linear attention (elu+1 feature map) on 8 Trainium2 NeuronCores.

Problem: B=2, L=2048, D=512, H=8, dh=64.
    Q/K/V = x @ W_{q,k,v};  Qf/Kf = elu(QK)+1  (Kf, V masked by ~pad)
    out_t = (sum_{s<=t} (Qf_t . Kf_s) V_s) / (Qf_t . sum_{s<=t} Kf_s + eps)
    y = concat_heads(out) @ W_o.T

Sharding: core c handles batch b = c//4 and head pair hp = c%4 (heads
2hp, 2hp+1).  Each core computes its 2 heads' attention over the full
sequence and a partial output projection through the matching 128-column
slice of W_o; partials of the 4 cores of each batch are summed during
unshard.

v4 (bf16, fused pipeline): all matmul operands bf16 (1 cyc/row vs 4 for
fp32), inputs converted host-side (halves DMA).  Single pool scope with
projection->attention work interleaved per 512-token L-chunk: no
phase-boundary pool release, so the PE streams continuously (p-state).
Attention chunk math produces O in NORMAL layout (tokens on
partitions): intra = matmul(lhsT=A^T, rhs=[V|1]), inter =
matmul(lhsT=Qf, rhs=[S|sk]); the ones-column in vn and the sk column in
the bf16 running state land the denominator in column 64 of the same
PSUM tile (no extra den matmuls, no cumsum scan, eps dropped since den
is O(30+)).  Division is a per-partition ACT scale on normal-layout O;
the two heads merge into one K=128 output projection per token tile.
phi(x) = min(exp(x),1) + relu(x): exp+relu on ACT over stacked Q/K
rows, min on the otherwise-idle GpSimd (SBUF-only engine: no PSUM
reads, no stt), two all-bf16 DVE adds (16-bit 2x rate).  Kf^T->Kf and
O->O^T via PE transposes (DMA transposes head-of-line-block the SP
queue; measured slower).  K=64 contractions run unpadded (bf16 K=64
streams fine, unlike fp32).
"""

import sys

for _p in ("/opt/trn_rl_repo", "/opt/pypackages"):
    if _p not in sys.path:
        sys.path.append(_p)

import numpy as np

B, L, D, H, DH = 2, 2048, 512, 8, 64
N_CORES = 8
P = 128
C = 128                 # attention chunk (tokens)
NCH = L // C            # 16 chunks
GRP = 2                 # chunks per A/den group (4 chunk-heads)
NLC = 4                 # projection L-chunks of 512
VSTR = 130              # V tile stride per token tile: (64 V + 1 one) x 2 heads
EPS = 1e-6              # kept for test.py's npsim; device drops it (den >> eps)

_CACHE = {}


def _build(apply_mask: bool, use_gps: bool = True):
    import concourse.bacc as bacc
    import concourse.mybir as mybir
    import concourse.tile as tile

    f32 = mybir.dt.float32
    bf16 = mybir.dt.bfloat16
    Alu = mybir.AluOpType
    Act = mybir.ActivationFunctionType

    nc = bacc.Bacc("TRN2", target_bir_lowering=False, debug=False,
                   num_devices=N_CORES)

    xT_d = nc.dram_tensor("xT", [4, P, L], bf16, kind="ExternalInput").ap()
    wqk_d = nc.dram_tensor("wqk", [P, 1024], bf16, kind="ExternalInput").ap()
    wv_d = nc.dram_tensor("wv", [P, 512], bf16, kind="ExternalInput").ap()
    wo_d = nc.dram_tensor("wo", [P, 512], bf16, kind="ExternalInput").ap()
    msk_d = nc.dram_tensor("msk", [P, 512], bf16, kind="ExternalInput").ap()
    idn_d = nc.dram_tensor("idn", [P, P], bf16, kind="ExternalInput").ap()
    if apply_mask:
        mc_d = nc.dram_tensor("mcol", [P, NCH], f32, kind="ExternalInput").ap()
    part_d = nc.dram_tensor("part", [L, 512], bf16, kind="ExternalOutput").ap()

    with tile.TileContext(nc) as tc:
        with tc.tile_pool(name="persist", bufs=1) as pp, \
             tc.tile_pool(name="big", bufs=3, space="PSUM") as bigp, \
             tc.tile_pool(name="on", bufs=2, space="PSUM") as onp, \
             tc.tile_pool(name="upd", bufs=1, space="PSUM") as updp, \
             tc.tile_pool(name="tr", bufs=2, space="PSUM") as trp, \
             tc.tile_pool(name="wk", bufs=3) as wk:
            gp = nc.gpsimd if use_gps else nc.vector
            xt = [[pp.tile([P, 512], bf16, tag=f"xt{d}_{lc}",
                           name=f"xt{d}_{lc}") for lc in range(NLC)]
                  for d in range(4)]
            wqk = pp.tile([P, 1024], bf16, tag="wqk")
            wv = pp.tile([P, 512], bf16, tag="wv")
            wo = pp.tile([P, 512], bf16, tag="wo")
            msk = pp.tile([P, 512], bf16, tag="msk")
            idn = pp.tile([P, P], bf16, tag="idn")
            qf = [pp.tile([64, 1024], bf16, tag=f"qf{lc}", name=f"qf{lc}")
                  for lc in range(NLC)]
            kf = [pp.tile([64, 1024], bf16, tag=f"kf{lc}", name=f"kf{lc}")
                  for lc in range(NLC)]
            vn = [pp.tile([P, 4 * VSTR], bf16, tag=f"vn{lc}", name=f"vn{lc}")
                  for lc in range(NLC)]
            kn = [pp.tile([P, 512], bf16, tag=f"kn{lc}", name=f"kn{lc}")
                  for lc in range(NLC)]
            ot = [pp.tile([P, 512], bf16, tag=f"ot{lc}", name=f"ot{lc}")
                  for lc in range(NLC)]
            s_sb = pp.tile([64, 130], bf16, tag="s_sb")  # head h at cols h*65
            if apply_mask:
                mc = pp.tile([P, NCH], f32, tag="mc")

            # wqk + lc0's x first so the first projection starts early
            nc.sync.dma_start(out=wqk[:], in_=wqk_d[:])
            for d in range(4):
                nc.sync.dma_start(out=xt[d][0][:], in_=xT_d[d, :, 0:512])
            nc.sync.dma_start(out=wv[:], in_=wv_d[:])
            nc.sync.dma_start(out=msk[:], in_=msk_d[:])
            nc.sync.dma_start(out=idn[:], in_=idn_d[:])
            nc.sync.dma_start(out=wo[:], in_=wo_d[:])
            for lc in range(1, NLC):
                for d in range(4):
                    nc.sync.dma_start(
                        out=xt[d][lc][:],
                        in_=xT_d[d, :, lc * 512:(lc + 1) * 512])
            if apply_mask:
                nc.sync.dma_start(out=mc[:], in_=mc_d[:])

            # vn[lc] viewed as [P, k, h, 65]; col 64 of each block = 1.0
            vn4 = [t[:].rearrange("p (t h c) -> p t h c", t=4, h=2, c=65)
                   for t in vn]
            for lc in range(NLC):
                nc.vector.memset(vn4[lc][:, :, :, 64:65], 1.0)
            nc.vector.memset(s_sb[:], 0.0)
            qf2 = [t[:].rearrange("p (h t) -> p h t", h=2) for t in qf]
            kf2 = [t[:].rearrange("p (h t) -> p h t", h=2) for t in kf]

            def emit_proj(lc):
                for h in (0, 1):
                    # one MM streams xt once for BOTH Q_h (rows 0:64)
                    # and K_h (rows 64:128)
                    ps = bigp.tile([P, 512], f32, tag="big", name="qkps")
                    for d in range(4):
                        base = d * 256 + h * 128
                        nc.tensor.matmul(
                            ps[:], lhsT=wqk[:, base:base + 128],
                            rhs=xt[d][lc][:], start=(d == 0), stop=(d == 3))
                    hs512 = slice(h * 512, (h + 1) * 512)
                    # phi(x) = min(exp(x),1) + relu(x); exp/relu on ACT over
                    # stacked Q/K rows, min on GpSimd, all-bf16 DVE adds
                    et = wk.tile([P, 512], bf16, tag="e")
                    nc.scalar.activation(et[:], ps[:], Act.Exp)
                    rl = wk.tile([P, 512], bf16, tag="r")
                    nc.scalar.activation(rl[:], ps[:], Act.Relu)
                    mt = wk.tile([P, 512], bf16, tag="m")
                    gp.tensor_scalar_min(mt[:], et[:], 1.0)
                    nc.vector.tensor_add(
                        qf[lc][:, hs512], mt[0:64, :], rl[0:64, :])
                    nc.vector.tensor_add(
                        kf[lc][:, hs512], mt[64:128, :], rl[64:128, :])
                    # Kf normal layout via PE transpose
                    tr_ps = trp.tile([P, 256], bf16, tag="tr")
                    for k in range(4):
                        nc.tensor.transpose(
                            tr_ps[:, k * 64:(k + 1) * 64],
                            kf2[lc][:, h, k * P:(k + 1) * P],
                            idn[0:64, 0:64])
                    kn2 = kn[lc][:].rearrange("p (k h e) -> p k h e", k=4, h=2)
                    if apply_mask:
                        for k in range(4):
                            ti = lc * 4 + k
                            nc.vector.tensor_scalar_mul(
                                kn2[:, k, h], tr_ps[:, k * 64:(k + 1) * 64],
                                mc[:, ti:ti + 1])
                    else:
                        nc.vector.tensor_copy(kn2[:, :, h], tr_ps[:])
                # V projection (normal layout) + copy into vn
                v_ps = bigp.tile([P, 512], f32, tag="big", name="vps")
                for k in range(4):
                    for d in range(4):
                        nc.tensor.matmul(
                            v_ps[:, k * P:(k + 1) * P],
                            lhsT=xt[d][lc][:, k * P:(k + 1) * P],
                            rhs=wv[:, d * P:(d + 1) * P],
                            start=(d == 0), stop=(d == 3))
                v_src = v_ps[:].rearrange("p (k h e) -> p k h e", k=4, h=2)
                if apply_mask:
                    for k in range(4):
                        ti = lc * 4 + k
                        nc.vector.tensor_scalar_mul(
                            vn4[lc][:, k, :, 0:64], v_src[:, k],
                            mc[:, ti:ti + 1])
                else:
                    nc.scalar.activation(vn4[lc][:, :, :, 0:64], v_src,
                                         Act.Copy)

            def emit_group(g):
                lc = (g * GRP) // 4
                chunks = [g * GRP + u for u in range(GRP)]
                jhs = [(i, h) for i in chunks for h in (0, 1)]
                # A^T for the group's 4 chunk-heads
                a_ps = bigp.tile([P, 512], f32, tag="big", name="aps")
                for j, (i, h) in enumerate(jhs):
                    cs = slice((i % 4) * C, (i % 4 + 1) * C)
                    nc.tensor.matmul(
                        a_ps[:, j * P:(j + 1) * P],
                        lhsT=kf2[lc][:, h, cs], rhs=qf2[lc][:, h, cs],
                        start=True, stop=True)
                am = wk.tile([P, 512], bf16, tag="am")
                if apply_mask:
                    for u, i in enumerate(chunks):
                        nc.vector.scalar_tensor_tensor(
                            am[:, u * 256:(u + 1) * 256],
                            a_ps[:, u * 256:(u + 1) * 256],
                            mc[:, i:i + 1], msk[:, 0:256],
                            op0=Alu.mult, op1=Alu.mult)
                else:
                    nc.vector.tensor_mul(am[:], a_ps[:], msk[:])
                # O in NORMAL layout [tokens, e] + den in col 64
                on_ps = onp.tile([P, 2 * GRP * 65], f32, tag="on")
                upd_ps = updp.tile([64, 2 * GRP * 65], f32, tag="upd")
                for u, i in enumerate(chunks):
                    k = i % 4
                    cs = slice(k * C, (k + 1) * C)
                    for h in (0, 1):
                        j = 2 * u + h
                        os_ = slice(j * 65, j * 65 + 65)
                        vb = k * VSTR
                        vh1 = vn[lc][:, vb + 65 * h:vb + 65 * h + 65]
                        # intra: Sum_s A^T[s,t] [V|1][s,:]
                        nc.tensor.matmul(
                            on_ps[:, os_], lhsT=am[:, j * P:(j + 1) * P],
                            rhs=vh1, start=True, stop=(i == 0))
                        # inter: Sum_e Qf[e,t] [S|sk][e,:]
                        if i > 0:
                            nc.tensor.matmul(
                                on_ps[:, os_], lhsT=qf2[lc][:, h, cs],
                                rhs=s_sb[:, h * 65:h * 65 + 65],
                                start=False, stop=True)
                        # state update for this chunk-head
                        nc.tensor.matmul(
                            upd_ps[:, os_],
                            lhsT=kn[lc][:, k * P + 64 * h:k * P + 64 * h + 64],
                            rhs=vh1, start=True, stop=True)
                    # fold this chunk's update into the running state
                    # (next chunk's inter/den-inter depend on it)
                    nc.vector.tensor_add(
                        s_sb[:], s_sb[:], upd_ps[:, u * 130:(u + 1) * 130])
                # batched 1/den over the group's 4 den columns
                on4 = on_ps[:].rearrange("p (j c) -> p j c", c=65)
                rcl = wk.tile([P, 2 * GRP], f32, tag="rcl")
                nc.vector.reciprocal(rcl[:], on4[:, :, 64])
                # divide + transpose back to O^T (merged heads), project
                for u, i in enumerate(chunks):
                    cs = slice((i % 4) * C, (i % 4 + 1) * C)
                    onf = wk.tile([P, P], bf16, tag="onf")
                    for h in (0, 1):
                        j = 2 * u + h
                        nc.scalar.activation(
                            onf[:, 64 * h:64 * h + 64], on4[:, j, 0:64],
                            Act.Copy, scale=rcl[:, j:j + 1])
                    t2 = trp.tile([P, 256], bf16, tag="tr", name="t2")
                    nc.tensor.transpose(t2[:, 0:P], onf[:], idn[:])
                    nc.vector.tensor_copy(ot[lc][:, cs], t2[:, 0:P])
                    ps0 = bigp.tile([P, 512], f32, tag="big", name="prj")
                    nc.tensor.matmul(ps0[:], lhsT=ot[lc][:, cs], rhs=wo[:],
                                     start=True, stop=True)
                    osb = wk.tile([P, 512], bf16, tag="osb")
                    if u == 0:
                        nc.scalar.activation(osb[:], ps0[:], Act.Copy)
                    else:
                        nc.vector.tensor_copy(osb[:], ps0[:])
                    nc.sync.dma_start(out=part_d[i * P:(i + 1) * P, :],
                                      in_=osb[:])

            for lc in range(NLC):
                emit_proj(lc)
                for gg in range(2):
                    emit_group(lc * 2 + gg)

    nc.compile()
    return nc


def _get_program(apply_mask: bool):
    key = bool(apply_mask)
    if key not in _CACHE:
        from concourse.bass_interp import get_hw_module
        nc = _build(key)
        nc.m = get_hw_module(nc.m)
        _CACHE[key] = nc
    return _CACHE[key]


def _in_maps(x, key_padding_mask, W_q, W_k, W_v, W_o, apply_mask):
    import ml_dtypes
    bf = ml_dtypes.bfloat16
    triu = np.triu(np.ones((P, P), np.float32))
    msk = np.tile(triu, (1, 4)).astype(bf)
    maps = []
    for c in range(N_CORES):
        b, hp = divmod(c, 4)
        xT = np.ascontiguousarray(x[b].T).reshape(4, P, L).astype(bf)

        def wslice(W):
            w = W[:, 2 * hp:2 * hp + 2, :].reshape(D, P)
            return np.ascontiguousarray(
                w.reshape(4, P, P).transpose(1, 0, 2).reshape(P, 512))

        a = wslice(W_q).reshape(P, 4, 2, 64)
        bqk = wslice(W_k).reshape(P, 4, 2, 64)
        wqk = np.ascontiguousarray(
            np.stack([a, bqk], axis=3).reshape(P, 1024)).astype(bf)
        # wo: rows 0:64 = head 2hp dims, rows 64:128 = head 2hp+1 dims
        wo = np.ascontiguousarray(
            W_o[:, P * hp:P * (hp + 1)].T).astype(bf)
        m = {"xT": xT, "wqk": wqk, "wv": wslice(W_v).astype(bf),
             "wo": wo, "msk": msk, "idn": np.eye(P, dtype=bf)}
        if apply_mask:
            keep = (~key_padding_mask[b]).astype(np.float32)  # (L,)
            m["mcol"] = np.ascontiguousarray(keep.reshape(NCH, P).T)
        maps.append(m)
    return maps


def kernel(x, key_padding_mask, W_q, W_k, W_v, W_o, _trace=False):
    from concourse.bass_utils import run_bass_kernel_spmd

    x = np.asarray(x, dtype=np.float32)
    key_padding_mask = np.asarray(key_padding_mask).astype(bool)
    apply_mask = bool(key_padding_mask.any())
    nc = _get_program(apply_mask)
    maps = _in_maps(x, key_padding_mask, np.asarray(W_q, np.float32),
                    np.asarray(W_k, np.float32), np.asarray(W_v, np.float32),
                    np.asarray(W_o, np.float32), apply_mask)
    res = run_bass_kernel_spmd(nc, maps, core_ids=list(range(N_CORES)),
                               trace=_trace)
    kernel.last_results = res
    out = np.zeros((B, L, D), np.float32)
    for c in range(N_CORES):
        out[c // 4] += np.asarray(res.results[c]["part"], np.float32)
    return out


# revision 18
# speedup vs baseline: 1.4760x; 1.4760x over previous
"""Causal linear attention (elu+1 feature map) on 8 Trainium2 NeuronCores.

Problem: B=2, L=2048, D=512, H=8, dh=64.
    Q/K/V = x @ W_{q,k,v};  Qf/Kf = elu(QK)+1  (Kf, V masked by ~pad)
    out_t = (sum_{s<=t} (Qf_t . Kf_s) V_s) / (Qf_t . sum_{s<=t} Kf_s + eps)
    y = concat_heads(out) @ W_o.T

Sharding: core c handles batch b = c//4 and head pair hp = c%4 (heads
2hp, 2hp+1).  Each core computes its 2 heads' attention over the full
sequence and a partial output projection through the matching 128-column
slice of W_o; partials of the 4 cores of each batch are summed during
unshard.

v4 (bf16, fused pipeline): all matmul operands bf16 (1 cyc/row vs 4 for
fp32), inputs converted host-side (halves DMA).  Single pool scope with
projection->attention work interleaved per 512-token L-chunk: no
phase-boundary pool release, so the PE streams continuously (p-state).
Attention chunk math produces O in NORMAL layout (tokens on
partitions): intra = matmul(lhsT=A^T, rhs=[V|1]), inter =
matmul(lhsT=Qf, rhs=[S|sk]); the ones-column in vn and the sk column in
the bf16 running state land the denominator in column 64 of the same
PSUM tile (no extra den matmuls, no cumsum scan, eps dropped since den
is O(30+)).  Division is a per-partition ACT scale on normal-layout O;
the two heads merge into one K=128 output projection per token tile.
phi(x) = min(exp(x),1) + relu(x): exp+relu on ACT over stacked Q/K
rows, min on the otherwise-idle GpSimd (SBUF-only engine: no PSUM
reads, no stt), two all-bf16 DVE adds (16-bit 2x rate).  Kf^T->Kf and
O->O^T via PE transposes (DMA transposes head-of-line-block the SP
queue; measured slower).  K=64 contractions run unpadded (bf16 K=64
streams fine, unlike fp32).
"""

import sys

for _p in ("/opt/trn_rl_repo", "/opt/pypackages"):
    if _p not in sys.path:
        sys.path.append(_p)

import numpy as np

B, L, D, H, DH = 2, 2048, 512, 8, 64
N_CORES = 8
P = 128
C = 128                 # attention chunk (tokens)
NCH = L // C            # 16 chunks
GRP = 2                 # chunks per A/den group (4 chunk-heads)
NLC = 4                 # projection L-chunks of 512
VSTR = 130              # V tile stride per token tile: (64 V + 1 one) x 2 heads
EPS = 1e-6              # kept for test.py's npsim; device drops it (den >> eps)

_CACHE = {}


def _build(apply_mask: bool, use_gps: bool = True):
    import concourse.bacc as bacc
    import concourse.mybir as mybir
    import concourse.tile as tile

    f32 = mybir.dt.float32
    bf16 = mybir.dt.bfloat16
    Alu = mybir.AluOpType
    Act = mybir.ActivationFunctionType

    nc = bacc.Bacc("TRN2", target_bir_lowering=False, debug=False,
                   num_devices=N_CORES)

    xT_d = nc.dram_tensor("xT", [4, P, L], bf16, kind="ExternalInput").ap()
    wqk_d = nc.dram_tensor("wqk", [P, 1024], bf16, kind="ExternalInput").ap()
    wv_d = nc.dram_tensor("wv", [P, 512], bf16, kind="ExternalInput").ap()
    wo_d = nc.dram_tensor("wo", [P, 512], bf16, kind="ExternalInput").ap()
    msk_d = nc.dram_tensor("msk", [P, 512], bf16, kind="ExternalInput").ap()
    idn_d = nc.dram_tensor("idn", [P, P], bf16, kind="ExternalInput").ap()
    if apply_mask:
        mc_d = nc.dram_tensor("mcol", [P, NCH], f32, kind="ExternalInput").ap()
    part_d = nc.dram_tensor("part", [L, 512], bf16, kind="ExternalOutput").ap()

    with tile.TileContext(nc) as tc:
        with tc.tile_pool(name="persist", bufs=1) as pp, \
             tc.tile_pool(name="big", bufs=3, space="PSUM") as bigp, \
             tc.tile_pool(name="on", bufs=2, space="PSUM") as onp, \
             tc.tile_pool(name="upd", bufs=1, space="PSUM") as updp, \
             tc.tile_pool(name="tr", bufs=2, space="PSUM") as trp, \
             tc.tile_pool(name="wk", bufs=3) as wk:
            gp = nc.gpsimd if use_gps else nc.vector
            xt = [[pp.tile([P, 512], bf16, tag=f"xt{d}_{lc}",
                           name=f"xt{d}_{lc}") for lc in range(NLC)]
                  for d in range(4)]
            wqk = pp.tile([P, 1024], bf16, tag="wqk")
            wv = pp.tile([P, 512], bf16, tag="wv")
            wo = pp.tile([P, 512], bf16, tag="wo")
            msk = pp.tile([P, 512], bf16, tag="msk")
            idn = pp.tile([P, P], bf16, tag="idn")
            qf = [pp.tile([64, 1024], bf16, tag=f"qf{lc}", name=f"qf{lc}")
                  for lc in range(NLC)]
            kf = [pp.tile([64, 1024], bf16, tag=f"kf{lc}", name=f"kf{lc}")
                  for lc in range(NLC)]
            vn = [pp.tile([P, 4 * VSTR], bf16, tag=f"vn{lc}", name=f"vn{lc}")
                  for lc in range(NLC)]
            kn = [pp.tile([P, 512], bf16, tag=f"kn{lc}", name=f"kn{lc}")
                  for lc in range(NLC)]
            ot = [pp.tile([P, 512], bf16, tag=f"ot{lc}", name=f"ot{lc}")
                  for lc in range(NLC)]
            s_sb = pp.tile([64, 130], bf16, tag="s_sb")  # head h at cols h*65
            if apply_mask:
                mc = pp.tile([P, NCH], f32, tag="mc")

            # wqk + lc0's x first so the first projection starts early
            nc.sync.dma_start(out=wqk[:], in_=wqk_d[:])
            for d in range(4):
                nc.sync.dma_start(out=xt[d][0][:], in_=xT_d[d, :, 0:512])
            nc.sync.dma_start(out=wv[:], in_=wv_d[:])
            nc.sync.dma_start(out=msk[:], in_=msk_d[:])
            nc.sync.dma_start(out=idn[:], in_=idn_d[:])
            nc.sync.dma_start(out=wo[:], in_=wo_d[:])
            for lc in range(1, NLC):
                for d in range(4):
                    nc.sync.dma_start(
                        out=xt[d][lc][:],
                        in_=xT_d[d, :, lc * 512:(lc + 1) * 512])
            if apply_mask:
                nc.sync.dma_start(out=mc[:], in_=mc_d[:])

            # vn[lc] viewed as [P, k, h, 65]; col 64 of each block = 1.0
            vn4 = [t[:].rearrange("p (t h c) -> p t h c", t=4, h=2, c=65)
                   for t in vn]
            for lc in range(NLC):
                nc.vector.memset(vn4[lc][:, :, :, 64:65], 1.0)
            nc.vector.memset(s_sb[:], 0.0)
            qf2 = [t[:].rearrange("p (h t) -> p h t", h=2) for t in qf]
            kf2 = [t[:].rearrange("p (h t) -> p h t", h=2) for t in kf]

            def emit_proj(lc):
                for h in (0, 1):
                    # one MM streams xt once for BOTH Q_h (rows 0:64)
                    # and K_h (rows 64:128)
                    ps = bigp.tile([P, 512], f32, tag="big", name="qkps")
                    for d in range(4):
                        base = d * 256 + h * 128
                        nc.tensor.matmul(
                            ps[:], lhsT=wqk[:, base:base + 128],
                            rhs=xt[d][lc][:], start=(d == 0), stop=(d == 3))
                    hs512 = slice(h * 512, (h + 1) * 512)
                    # phi(x) = min(exp(x),1) + relu(x); exp/relu on ACT over
                    # stacked Q/K rows, min on GpSimd, all-bf16 DVE adds
                    et = wk.tile([P, 512], bf16, tag="e")
                    nc.scalar.activation(et[:], ps[:], Act.Exp)
                    rl = wk.tile([P, 512], bf16, tag="r")
                    nc.scalar.activation(rl[:], ps[:], Act.Relu)
                    mt = wk.tile([P, 512], bf16, tag="m")
                    gp.tensor_scalar_min(mt[:], et[:], 1.0)
                    nc.vector.tensor_add(
                        qf[lc][:, hs512], mt[0:64, :], rl[0:64, :])
                    nc.vector.tensor_add(
                        kf[lc][:, hs512], mt[64:128, :], rl[64:128, :])
                    # Kf normal layout via PE transpose
                    tr_ps = trp.tile([P, 256], bf16, tag="tr")
                    for k in range(4):
                        nc.tensor.transpose(
                            tr_ps[:, k * 64:(k + 1) * 64],
                            kf2[lc][:, h, k * P:(k + 1) * P],
                            idn[0:64, 0:64])
                    kn2 = kn[lc][:].rearrange("p (k h e) -> p k h e", k=4, h=2)
                    if apply_mask:
                        for k in range(4):
                            ti = lc * 4 + k
                            nc.vector.tensor_scalar_mul(
                                kn2[:, k, h], tr_ps[:, k * 64:(k + 1) * 64],
                                mc[:, ti:ti + 1])
                    else:
                        nc.vector.tensor_copy(kn2[:, :, h], tr_ps[:])
                # V projection (normal layout) + copy into vn
                v_ps = bigp.tile([P, 512], f32, tag="big", name="vps")
                for k in range(4):
                    for d in range(4):
                        nc.tensor.matmul(
                            v_ps[:, k * P:(k + 1) * P],
                            lhsT=xt[d][lc][:, k * P:(k + 1) * P],
                            rhs=wv[:, d * P:(d + 1) * P],
                            start=(d == 0), stop=(d == 3))
                v_src = v_ps[:].rearrange("p (k h e) -> p k h e", k=4, h=2)
                if apply_mask:
                    for k in range(4):
                        ti = lc * 4 + k
                        nc.vector.tensor_scalar_mul(
                            vn4[lc][:, k, :, 0:64], v_src[:, k],
                            mc[:, ti:ti + 1])
                else:
                    nc.scalar.activation(vn4[lc][:, :, :, 0:64], v_src,
                                         Act.Copy)

            def emit_group(g):
                lc = (g * GRP) // 4
                chunks = [g * GRP + u for u in range(GRP)]
                jhs = [(i, h) for i in chunks for h in (0, 1)]
                # A^T for the group's 4 chunk-heads
                a_ps = bigp.tile([P, 512], f32, tag="big", name="aps")
                for j, (i, h) in enumerate(jhs):
                    cs = slice((i % 4) * C, (i % 4 + 1) * C)
                    nc.tensor.matmul(
                        a_ps[:, j * P:(j + 1) * P],
                        lhsT=kf2[lc][:, h, cs], rhs=qf2[lc][:, h, cs],
                        start=True, stop=True)
                am = wk.tile([P, 512], bf16, tag="am")
                if apply_mask:
                    for u, i in enumerate(chunks):
                        nc.vector.scalar_tensor_tensor(
                            am[:, u * 256:(u + 1) * 256],
                            a_ps[:, u * 256:(u + 1) * 256],
                            mc[:, i:i + 1], msk[:, 0:256],
                            op0=Alu.mult, op1=Alu.mult)
                else:
                    nc.vector.tensor_mul(am[:], a_ps[:], msk[:])
                # O in NORMAL layout [tokens, e] + den in col 64
                on_ps = onp.tile([P, 2 * GRP * 65], f32, tag="on")
                upd_ps = updp.tile([64, 2 * GRP * 65], f32, tag="upd")
                for u, i in enumerate(chunks):
                    k = i % 4
                    cs = slice(k * C, (k + 1) * C)
                    for h in (0, 1):
                        j = 2 * u + h
                        os_ = slice(j * 65, j * 65 + 65)
                        vb = k * VSTR
                        vh1 = vn[lc][:, vb + 65 * h:vb + 65 * h + 65]
                        # intra: Sum_s A^T[s,t] [V|1][s,:]
                        nc.tensor.matmul(
                            on_ps[:, os_], lhsT=am[:, j * P:(j + 1) * P],
                            rhs=vh1, start=True, stop=(i == 0))
                        # inter: Sum_e Qf[e,t] [S|sk][e,:]
                        if i > 0:
                            nc.tensor.matmul(
                                on_ps[:, os_], lhsT=qf2[lc][:, h, cs],
                                rhs=s_sb[:, h * 65:h * 65 + 65],
                                start=False, stop=True)
                        # state update for this chunk-head
                        nc.tensor.matmul(
                            upd_ps[:, os_],
                            lhsT=kn[lc][:, k * P + 64 * h:k * P + 64 * h + 64],
                            rhs=vh1, start=True, stop=True)
                    # fold this chunk's update into the running state
                    # (next chunk's inter/den-inter depend on it)
                    nc.vector.tensor_add(
                        s_sb[:], s_sb[:], upd_ps[:, u * 130:(u + 1) * 130])
                # batched 1/den over the group's 4 den columns
                on4 = on_ps[:].rearrange("p (j c) -> p j c", c=65)
                rcl = wk.tile([P, 2 * GRP], f32, tag="rcl")
                nc.vector.reciprocal(rcl[:], on4[:, :, 64])
                # divide + transpose back to O^T (merged heads), project
                for u, i in enumerate(chunks):
                    cs = slice((i % 4) * C, (i % 4 + 1) * C)
                    onf = wk.tile([P, P], bf16, tag="onf")
                    for h in (0, 1):
                        j = 2 * u + h
                        nc.scalar.activation(
                            onf[:, 64 * h:64 * h + 64], on4[:, j, 0:64],
                            Act.Copy, scale=rcl[:, j:j + 1])
                    t2 = trp.tile([P, 256], bf16, tag="tr", name="t2")
                    nc.tensor.transpose(t2[:, 0:P], onf[:], idn[:])
                    nc.vector.tensor_copy(ot[lc][:, cs], t2[:, 0:P])
                    ps0 = bigp.tile([P, 512], f32, tag="big", name="prj")
                    nc.tensor.matmul(ps0[:], lhsT=ot[lc][:, cs], rhs=wo[:],
                                     start=True, stop=True)
                    osb = wk.tile([P, 512], bf16, tag="osb")
                    if u == 0:
                        nc.scalar.activation(osb[:], ps0[:], Act.Copy)
                    else:
                        nc.vector.tensor_copy(osb[:], ps0[:])
                    nc.sync.dma_start(out=part_d[i * P:(i + 1) * P, :],
                                      in_=osb[:])

            for lc in range(NLC):
                emit_proj(lc)
                for gg in range(2):
                    emit_group(lc * 2 + gg)

    nc.compile()
    return nc


def _get_program(apply_mask: bool):
    key = bool(apply_mask)
    if key not in _CACHE:
        from concourse.bass_interp import get_hw_module
        nc = _build(key, use_gps=False)
        nc.m = get_hw_module(nc.m)
        _CACHE[key] = nc
    return _CACHE[key]


def _in_maps(x, key_padding_mask, W_q, W_k, W_v, W_o, apply_mask):
    import ml_dtypes
    bf = ml_dtypes.bfloat16
    triu = np.triu(np.ones((P, P), np.float32))
    msk = np.tile(triu, (1, 4)).astype(bf)
    maps = []
    for c in range(N_CORES):
        b, hp = divmod(c, 4)
        xT = np.ascontiguousarray(x[b].T).reshape(4, P, L).astype(bf)

        def wslice(W):
            w = W[:, 2 * hp:2 * hp + 2, :].reshape(D, P)
            return np.ascontiguousarray(
                w.reshape(4, P, P).transpose(1, 0, 2).reshape(P, 512))

        a = wslice(W_q).reshape(P, 4, 2, 64)
        bqk = wslice(W_k).reshape(P, 4, 2, 64)
        wqk = np.ascontiguousarray(
            np.stack([a, bqk], axis=3).reshape(P, 1024)).astype(bf)
        # wo: rows 0:64 = head 2hp dims, rows 64:128 = head 2hp+1 dims
        wo = np.ascontiguousarray(
            W_o[:, P * hp:P * (hp + 1)].T).astype(bf)
        m = {"xT": xT, "wqk": wqk, "wv": wslice(W_v).astype(bf),
             "wo": wo, "msk": msk, "idn": np.eye(P, dtype=bf)}
        if apply_mask:
            keep = (~key_padding_mask[b]).astype(np.float32)  # (L,)
            m["mcol"] = np.ascontiguousarray(keep.reshape(NCH, P).T)
        maps.append(m)
    return maps


def kernel(x, key_padding_mask, W_q, W_k, W_v, W_o, _trace=False):
    from concourse.bass_utils import run_bass_kernel_spmd

    x = np.asarray(x, dtype=np.float32)
    key_padding_mask = np.asarray(key_padding_mask).astype(bool)
    apply_mask = bool(key_padding_mask.any())
    nc = _get_program(apply_mask)
    maps = _in_maps(x, key_padding_mask, np.asarray(W_q, np.float32),
                    np.asarray(W_k, np.float32), np.asarray(W_v, np.float32),
                    np.asarray(W_o, np.float32), apply_mask)
    res = run_bass_kernel_spmd(nc, maps, core_ids=list(range(N_CORES)),
                               trace=_trace)
    kernel.last_results = res
    out = np.zeros((B, L, D), np.float32)
    for c in range(N_CORES):
        out[c // 4] += np.asarray(res.results[c]["part"], np.float32)
    return out
